# revision 13
# baseline (speedup 1.0000x reference)
"""BiLSTM-CRF Trainium2 kernel (8 NeuronCores, SPMD).

Strategy (v2 — fp32 end-to-end, upload-lean):
 - Data-parallel over the sequence: core k owns tokens [1024k, 1024k+1024).
 - Chunked-warmup LSTM: 128 rows x 8 owned tokens each, W=40 warmup steps
   run in lockstep (state reconvergence ~1e-10; boundary rows at t=0 /
   t=T-1 get exact h0/c0 injection, so no approximation there at all).
 - Everything fp32 on device: embeddings, weights, h/c state, PSUM.
   sigmoid computed as 0.5 + 0.5*tanh(0.5 z) (device tanh spline is
   ~4 ULP; measured 4.4e-8 abs err for the composite, ~20x better than
   the direct sigmoid spline).
 - Upload diet (axon tunnel ~85 MB/s, ~60 ms fixed): one shared fp32
   embedding window per core serves both directions (1.14 MB); the
   4 MB of LSTM weights are uploaded as 1/8 row-slices per core and
   AllGathered on device; identity generated with affine_select;
   c0 injection tiles built on device from a 2x256 vector.
 - feats.T = W_out @ [h_f; h_b] + b_out in bulk; [16,1024] fp32 out/core.
 - Host: exact fp32 replica of the reference Viterbi scan (same op
   order as jax CPU; validated bit-exact) + vectorized backpointer
   replay + backtrack.
"""

import os
import sys
import time as _time

import numpy as np

sys.path.insert(0, "/opt/trn_rl_repo")

import concourse.bass as bass  # noqa: E402
import concourse.tile as tile  # noqa: E402
from concourse import bacc, mybir  # noqa: E402
from concourse.bass_utils import run_bass_kernel_spmd  # noqa: E402

# ---- problem constants (hardcoded per the task contract) ----
T = 8192
VOCAB = 100000
EMBED = 256
H = 256            # per-direction hidden
G4 = 1024
NT = 16
START_IX = 14
STOP_IX = 15
NEG = -10000.0
NCORES = 8
OWN = T // NCORES  # 1024

# chunked-warmup geometry
L = 8              # owned tokens per row
W = 40             # LSTM warmup steps per row
SL = L + W         # 48 lockstep steps
NCOL = 8 * 139     # 1112 emb/hs columns; col c <-> t_rel = c - (W+1)
COFF = W + 1       # 41
NF = 1024          # feats per core

FP32 = mybir.dt.float32

# gate reorder: torch [i,f,g,o] -> device [i,f,o,g] (sigmoid block 0:768)
GATE_PERM = np.concatenate([
    np.arange(0, 256), np.arange(256, 512), np.arange(768, 1024),
    np.arange(512, 768)])

# injection events: fwd (core 0) rows 0..5 consume t=0 at step 40-8r;
# bwd (core 7) rows 122..127 consume t=T-1 at step 8r-976.
INJ_STEPS = (0, 8, 16, 24, 32, 40)

_COMPILED = None


def _build_program():
    nc = bacc.Bacc("TRN2", target_bir_lowering=False, debug=False,
                   num_devices=NCORES)

    def din(name, shape):
        return nc.dram_tensor(name, list(shape), FP32,
                              kind="ExternalInput").ap()

    emb = din("emb", [256, NCOL])
    aux = din("aux", [4, NCOL])
    wsl = din("wsl", [128, G4])
    waux = din("waux", [4, G4])
    wout = din("wout", [513, NT])
    c0m = din("c0m", [1, 2 * H])      # cols 0:256 fwd c0, 256:512 bwd c0
    oh = din("oh", [1, 12 * 128])     # event e -> cols [128e, 128e+128)

    feats_out = nc.dram_tensor("featsT", [NT, NF], FP32,
                               kind="ExternalOutput").ap()

    AL = mybir.AluOpType
    ACT = mybir.ActivationFunctionType

    with tile.TileContext(nc) as tc:
        import contextlib
        ctx = contextlib.ExitStack()
        with ctx:
            dram = ctx.enter_context(
                tc.tile_pool(name="dram", bufs=1, space="DRAM"))
            const = ctx.enter_context(tc.tile_pool(name="const", bufs=1))
            state = ctx.enter_context(tc.tile_pool(name="state", bufs=1))
            work = ctx.enter_context(tc.tile_pool(name="work", bufs=2))
            zp = ctx.enter_context(
                tc.tile_pool(name="zp", bufs=3, space="PSUM"))
            tp = ctx.enter_context(
                tc.tile_pool(name="tp", bufs=2, space="PSUM"))

            # ---- weight AllGather: [128,1024]/core -> [1024,1024] ----
            wg_in = dram.tile([128, G4], FP32)
            wg_out = dram.tile([8 * 128, G4], FP32)
            nc.gpsimd.dma_start(wg_in[:], wsl[:, :])
            nc.gpsimd.collective_compute(
                "AllGather", AL.bypass,
                replica_groups=[list(range(NCORES))],
                ins=[wg_in.opt()], outs=[wg_out.opt()])

            # gathered row layout: core q block at 128q; within block,
            # matrix m (wihf,whhf,wihb,whhb) rows [32m:32m+32] hold the
            # original rows [32q:32q+32].
            wt = {}
            for m, tag in enumerate(("wif", "whf", "wib", "whb")):
                t0 = const.tile([128, G4], FP32, tag=f"{tag}0")
                t1 = const.tile([128, G4], FP32, tag=f"{tag}1")
                for q in range(4):
                    nc.sync.dma_start(
                        t0[32 * q:32 * (q + 1), :],
                        wg_out[128 * q + 32 * m:128 * q + 32 * m + 32, :])
                    nc.sync.dma_start(
                        t1[32 * q:32 * (q + 1), :],
                        wg_out[128 * (q + 4) + 32 * m:128 * (q + 4) + 32 * m + 32, :])
                wt[tag] = (t0, t1)

            # ---- plain input loads ----
            ef0 = const.tile([128, NCOL], FP32, tag="ef0")
            ef1 = const.tile([128, NCOL], FP32, tag="ef1")
            nc.sync.dma_start(ef0[:], emb[0:128, :])
            nc.sync.dma_start(ef1[:], emb[128:256, :])
            auxf = const.tile([2, NCOL], FP32, tag="auxf")
            auxb = const.tile([2, NCOL], FP32, tag="auxb")
            nc.sync.dma_start(auxf[:], aux[0:2, :])
            nc.sync.dma_start(auxb[:], aux[2:4, :])
            wauxf = const.tile([2, G4], FP32, tag="wauxf")
            wauxb = const.tile([2, G4], FP32, tag="wauxb")
            nc.sync.dma_start(wauxf[:], waux[0:2, :])
            nc.sync.dma_start(wauxb[:], waux[2:4, :])
            wo = []
            for i in range(4):
                woi = const.tile([128, NT], FP32, tag=f"wo{i}")
                nc.sync.dma_start(woi[:], wout[128 * i:128 * (i + 1), :])
                wo.append(woi)
            wob = const.tile([1, NT], FP32, tag="wob")
            nc.sync.dma_start(wob[:], wout[512:513, :])
            c0t = const.tile([1, 2 * H], FP32, tag="c0t")
            nc.sync.dma_start(c0t[:], c0m[:, :])
            oht = const.tile([1, 12 * 128], FP32, tag="oht")
            nc.sync.dma_start(oht[:], oh[:, :])

            # ---- identity for PE transpose (affine_select diag) ----
            ones = const.tile([128, 128], FP32, tag="ones")
            idn = const.tile([128, 128], FP32, tag="idn")
            nc.vector.memset(ones[:], 1.0)
            nc.gpsimd.affine_select(
                out=idn[:], in_=ones[:], pattern=[[1, 128]],
                compare_op=AL.is_equal, fill=0.0,
                base=0, channel_multiplier=-1)

            # ---- c0 injection tiles: onehot(row) (x) c0[dir] ----
            inj = []
            for e in range(12):
                ps = zp.tile([128, H], FP32, tag="z")
                src = c0t[0:1, 0:H] if e < 6 else c0t[0:1, H:2 * H]
                nc.tensor.matmul(ps[:], oht[0:1, 128 * e:128 * (e + 1)], src,
                                 start=True, stop=True)
                it = const.tile([128, H], FP32, tag=f"inj{e}")
                nc.vector.tensor_copy(out=it[:], in_=ps[:])
                inj.append(it)

            # ---- persistent state ----
            hsf0 = state.tile([128, NCOL], FP32, tag="hsf0")
            hsf1 = state.tile([128, NCOL], FP32, tag="hsf1")
            hsb0 = state.tile([128, NCOL], FP32, tag="hsb0")
            hsb1 = state.tile([128, NCOL], FP32, tag="hsb1")
            cf = state.tile([128, H], FP32, tag="cf")
            cb = state.tile([128, H], FP32, tag="cb")
            for t in (hsf0, hsf1, hsb0, hsb1, cf, cb):
                nc.vector.memset(t[:], 0.0)

            def strided(tl, base):
                # cols {base + 8r, r=0..127} of a [p, 8*m] tile
                q, b = divmod(base, L)
                v = tl[:].rearrange("p (n k) -> p n k", k=L)
                return v[:, q:q + 128, b:b + 1]

            def lstm_step(s, emb_base, h_base, auxd, wauxd,
                          wih, whh, hs, c, inj_e):
                w0, w1 = wih
                g0, g1 = whh
                h0t, h1t = hs
                z = zp.tile([128, G4], FP32, tag="z")
                ktiles = [
                    (strided(ef0, emb_base), w0[:]),
                    (strided(ef1, emb_base), w1[:]),
                    (strided(auxd, emb_base), wauxd[:]),
                    (strided(h0t, h_base), g0[:]),
                    (strided(h1t, h_base), g1[:]),
                ]
                for ki, (lhs, wmat) in enumerate(ktiles):
                    first, last = ki == 0, ki == len(ktiles) - 1
                    for half in (0, 1):
                        sl = slice(512 * half, 512 * (half + 1))
                        nc.tensor.matmul(z[:, sl], lhs, wmat[:, sl],
                                         start=first, stop=last)
                # gates: sigmoid(x) = 0.5 + 0.5*tanh(0.5x)
                sg = work.tile([128, 768], FP32, tag="sg")
                gg = work.tile([128, H], FP32, tag="gg")
                nc.scalar.activation(sg[:], z[:, 0:768], ACT.Tanh, scale=0.5)
                nc.scalar.activation(gg[:], z[:, 768:1024], ACT.Tanh)
                if inj_e is not None:
                    # c0 joins the incoming state (f-gate scales it);
                    # all-zero tile on non-boundary cores -> exact no-op
                    nc.vector.tensor_tensor(out=c[:], in0=c[:],
                                            in1=inj[inj_e][:], op=AL.add)
                ig = work.tile([128, H], FP32, tag="ig")
                fg = work.tile([128, H], FP32, tag="fg")
                og = work.tile([128, H], FP32, tag="og")
                for dst, lo in ((ig, 0), (fg, H), (og, 2 * H)):
                    nc.vector.tensor_scalar(
                        out=dst[:], in0=sg[:, lo:lo + H],
                        scalar1=0.5, scalar2=0.5, op0=AL.mult, op1=AL.add)
                c1 = work.tile([128, H], FP32, tag="c1")
                c2 = work.tile([128, H], FP32, tag="c2")
                nc.vector.tensor_tensor(out=c1[:], in0=fg[:], in1=c[:],
                                        op=AL.mult)
                nc.vector.tensor_tensor(out=c2[:], in0=ig[:], in1=gg[:],
                                        op=AL.mult)
                nc.vector.tensor_tensor(out=c[:], in0=c1[:], in1=c2[:],
                                        op=AL.add)
                thc = work.tile([128, H], FP32, tag="thc")
                nc.scalar.activation(thc[:], c[:], ACT.Tanh)
                hp = work.tile([128, H], FP32, tag="hp")
                nc.vector.tensor_tensor(out=hp[:], in0=og[:], in1=thc[:],
                                        op=AL.mult)
                return hp

            for s in range(SL):
                # fwd event index e = r = (40-s)/8 -> inj rows 0..5
                inj_f = (40 - s) // 8 if s in INJ_STEPS else None
                hp_f = lstm_step(s, s + 1, s, auxf, wauxf,
                                 wt["wif"], wt["whf"], (hsf0, hsf1), cf,
                                 inj_f)
                for half, dst in ((0, hsf0), (1, hsf1)):
                    pt = tp.tile([128, 128], FP32, tag="pt")
                    nc.tensor.transpose(
                        pt[:], hp_f[:, 128 * half:128 * (half + 1)], idn[:])
                    nc.vector.tensor_copy(strided(dst, s + 1), pt[:])
                inj_b = 6 + s // 8 if s in INJ_STEPS else None
                hp_b = lstm_step(s, 2 * W + 8 - s, 2 * W + 9 - s,
                                 auxb, wauxb,
                                 wt["wib"], wt["whb"], (hsb0, hsb1), cb,
                                 inj_b)
                for half, dst in ((0, hsb0), (1, hsb1)):
                    pt = tp.tile([128, 128], FP32, tag="pt")
                    nc.tensor.transpose(
                        pt[:], hp_b[:, 128 * half:128 * (half + 1)], idn[:])
                    nc.vector.tensor_copy(strided(dst, 2 * W + 8 - s), pt[:])

            # ---- bulk feats: featsT[i, tau] ; hs col = tau + COFF ----
            fsb = state.tile([NT, NF], FP32, tag="fsb")
            fstep = 512
            for f0 in range(0, NF, fstep):
                n = min(fstep, NF - f0)
                fp = zp.tile([NT, n], FP32, tag="z")
                c0_, c1_ = COFF + f0, COFF + f0 + n
                nc.tensor.matmul(fp[:], wo[0][:], hsf0[:, c0_:c1_],
                                 start=True, stop=False)
                nc.tensor.matmul(fp[:], wo[1][:], hsf1[:, c0_:c1_],
                                 start=False, stop=False)
                nc.tensor.matmul(fp[:], wo[2][:], hsb0[:, c0_:c1_],
                                 start=False, stop=False)
                nc.tensor.matmul(fp[:], wo[3][:], hsb1[:, c0_:c1_],
                                 start=False, stop=False)
                nc.tensor.matmul(fp[:], wob[:], auxf[0:1, c0_:c1_],
                                 start=False, stop=True)
                nc.vector.tensor_copy(out=fsb[:, f0:f0 + n], in_=fp[:])
            nc.sync.dma_start(feats_out[:, :], fsb[:])

    nc.compile()
    return nc


def _prep_core(k, sentence, embed, wihf_t, whhf_t, wihb_t, whhb_t,
               b_f, b_b, wh0_f, wh0_b, W_out, b_out, c0):
    s_k = OWN * k
    t = s_k + np.arange(NCOL) - COFF
    valid = (t >= 0) & (t < T)
    tv = np.clip(t, 0, T - 1)
    emb = np.ascontiguousarray(embed[sentence[tv]].T)   # [256, NCOL]
    emb[:, ~valid] = 0.0

    aux = np.zeros((4, NCOL), dtype=np.float32)
    aux[0] = valid
    aux[1] = (t == 0)
    aux[2] = valid
    aux[3] = (t == T - 1)

    wsl = np.concatenate([m[32 * k:32 * (k + 1)] for m in
                          (wihf_t, whhf_t, wihb_t, whhb_t)], axis=0)

    waux = np.stack([b_f, wh0_f, b_b, wh0_b]).astype(np.float32)

    wout = np.zeros((513, NT), dtype=np.float32)
    wout[0:256] = W_out[:, 0:256].T
    wout[256:512] = W_out[:, 256:512].T
    wout[512] = b_out

    c0m = np.ascontiguousarray(c0.astype(np.float32).reshape(1, 2 * H))

    oh = np.zeros((1, 12 * 128), dtype=np.float32)
    if k == 0:
        for e in range(6):
            oh[0, 128 * e + e] = 1.0
    if k == NCORES - 1:
        for j in range(6):
            oh[0, 128 * (6 + j) + 122 + j] = 1.0

    return {"emb": emb, "aux": aux, "wsl": np.ascontiguousarray(wsl),
            "waux": waux, "wout": wout, "c0m": c0m, "oh": oh}


def _host_viterbi(feats, trans):
    """Exact fp32 replica of the reference Viterbi scan + backtrack."""
    Tn = feats.shape[0]
    feats = np.ascontiguousarray(feats, dtype=np.float32)
    trans = np.ascontiguousarray(trans, dtype=np.float32)
    fv = np.full(NT, NEG, np.float32)
    fv[START_IX] = 0.0
    fv_prev = np.empty((Tn, NT), np.float32)
    for t in range(Tn):
        fv_prev[t] = fv
        temp = (fv[None, :] + feats[t][:, None]) + trans
        fv = temp.max(1)
    # vectorized backpointer replay (same fp op order per element)
    temp_all = (fv_prev[:, None, :] + feats[:, :, None]) + trans[None]
    bps = temp_all.argmax(2)                            # [Tn, 16]
    fv = fv + trans[:, STOP_IX]
    idc = int(fv.argmax())
    path = np.empty(Tn, np.int64)
    path[Tn - 1] = idc
    for t in range(Tn - 2, -1, -1):
        path[t] = bps[t + 1][path[t + 1]]
    return path


def kernel(sentence, embed, w_ih_f, w_hh_f, b_ih_f, b_hh_f,
           w_ih_b, w_hh_b, b_ih_b, b_hh_b, W_out, b_out,
           transition, h0, c0):
    global _COMPILED
    sentence = np.asarray(sentence).astype(np.int64)
    embed = np.asarray(embed, dtype=np.float32)
    args = [np.asarray(a, dtype=np.float32) for a in
            (w_ih_f, w_hh_f, b_ih_f, b_hh_f, w_ih_b, w_hh_b, b_ih_b, b_hh_b,
             W_out, b_out, transition, h0, c0)]
    (w_ih_f, w_hh_f, b_ih_f, b_hh_f, w_ih_b, w_hh_b, b_ih_b, b_hh_b,
     W_out, b_out, transition, h0, c0) = args

    wihf_t = np.ascontiguousarray(w_ih_f.T[:, GATE_PERM])
    whhf_t = np.ascontiguousarray(w_hh_f.T[:, GATE_PERM])
    wihb_t = np.ascontiguousarray(w_ih_b.T[:, GATE_PERM])
    whhb_t = np.ascontiguousarray(w_hh_b.T[:, GATE_PERM])
    b_f = (b_ih_f + b_hh_f)[GATE_PERM]
    b_b = (b_ih_b + b_hh_b)[GATE_PERM]
    wh0_f = (w_hh_f @ h0[0])[GATE_PERM]
    wh0_b = (w_hh_b @ h0[1])[GATE_PERM]

    if _COMPILED is None:
        _COMPILED = _build_program()
    nc = _COMPILED

    in_maps = [
        _prep_core(k, sentence, embed, wihf_t, whhf_t, wihb_t, whhb_t,
                   b_f, b_b, wh0_f, wh0_b, W_out, b_out, c0)
        for k in range(NCORES)
    ]

    _t0 = _time.perf_counter()
    res = run_bass_kernel_spmd(nc, in_maps, core_ids=list(range(NCORES)),
                               trace=False)
    kernel.last_dispatch_wall_ns = int((_time.perf_counter() - _t0) * 1e9)
    kernel.last_exec_time_ns = getattr(res, "exec_time_ns", None)

    feats_full = np.empty((T, NT), dtype=np.float32)
    for k in range(NCORES):
        feats_full[OWN * k:OWN * (k + 1)] = res.results[k]["featsT"].T
    if os.environ.get("KERNEL_DEBUG_FEATS"):
        np.save("/tmp/feats_device.npy", feats_full)

    path = _host_viterbi(feats_full, transition)
    return path.astype(np.int32)


# revision 15
# speedup vs baseline: 2.0130x; 2.0130x over previous
"""BiLSTM-CRF Trainium2 kernel (8 NeuronCores, SPMD).

Strategy (v2 — fp32 end-to-end, upload-lean):
 - Data-parallel over the sequence: core k owns tokens [1024k, 1024k+1024).
 - Chunked-warmup LSTM: 128 rows x 8 owned tokens each, W=40 warmup steps
   run in lockstep (state reconvergence ~1e-10; boundary rows at t=0 /
   t=T-1 get exact h0/c0 injection, so no approximation there at all).
 - Everything fp32 on device: embeddings, weights, h/c state, PSUM.
   sigmoid computed as 0.5 + 0.5*tanh(0.5 z) (device tanh spline is
   ~4 ULP; measured 4.4e-8 abs err for the composite, ~20x better than
   the direct sigmoid spline).
 - Upload diet (axon tunnel ~85 MB/s, ~60 ms fixed): one shared fp32
   embedding window per core serves both directions (1.14 MB); the
   4 MB of LSTM weights are uploaded as 1/8 row-slices per core and
   AllGathered on device; identity generated with affine_select;
   c0 injection tiles built on device from a 2x256 vector.
 - feats.T = W_out @ [h_f; h_b] + b_out in bulk; [16,1024] fp32 out/core.
 - Host: exact fp32 replica of the reference Viterbi scan (same op
   order as jax CPU; validated bit-exact) + vectorized backpointer
   replay + backtrack.
"""

import os
import sys
import time as _time

import numpy as np

sys.path.insert(0, "/opt/trn_rl_repo")

import concourse.bass as bass  # noqa: E402
import concourse.tile as tile  # noqa: E402
from concourse import bacc, mybir  # noqa: E402
from concourse.bass_utils import run_bass_kernel_spmd  # noqa: E402

# ---- problem constants (hardcoded per the task contract) ----
T = 8192
VOCAB = 100000
EMBED = 256
H = 256            # per-direction hidden
G4 = 1024
NT = 16
START_IX = 14
STOP_IX = 15
NEG = -10000.0
NCORES = 8
OWN = T // NCORES  # 1024

# chunked-warmup geometry
L = 8              # owned tokens per row
W = 40             # LSTM warmup steps per row
SL = L + W         # 48 lockstep steps
NCOL = 8 * 139     # 1112 emb/hs columns; col c <-> t_rel = c - (W+1)
COFF = W + 1       # 41
NF = 1024          # feats per core

FP32 = mybir.dt.float32

# gate reorder: torch [i,f,g,o] -> device [i,f,o,g] (sigmoid block 0:768)
GATE_PERM = np.concatenate([
    np.arange(0, 256), np.arange(256, 512), np.arange(768, 1024),
    np.arange(512, 768)])

# injection events: fwd (core 0) rows 0..5 consume t=0 at step 40-8r;
# bwd (core 7) rows 122..127 consume t=T-1 at step 8r-976.
INJ_STEPS = (0, 8, 16, 24, 32, 40)

_COMPILED = None
_DISPATCH = {}


def _install_fast_pjrt():
    """Cache the jit(shard_map(bass_exec)) callable across calls.

    ``bass2jax.run_bass_via_pjrt`` rebuilds the jit wrapper on every
    invocation (a fresh closure forces a full jax retrace, ~130 ms) and
    materializes each sharded output once per core (redundant D2H
    fetches).  Execution still flows unchanged through
    ``run_bass_kernel_spmd`` -> ``_bass_exec_p`` -> PJRT; this only
    memoizes the host-side dispatch plumbing.
    """
    from concourse import bass2jax as b2j

    if getattr(b2j.run_bass_via_pjrt, "_fast", False):
        return
    orig = b2j.run_bass_via_pjrt

    import jax
    from jax.sharding import Mesh, PartitionSpec
    from jax.experimental.shard_map import shard_map

    def build(nc, n_cores):
        b2j.install_neuronx_cc_hook()
        partition_name = (nc.partition_id_tensor.name
                          if nc.partition_id_tensor else None)
        in_names, out_names, out_avals, zero_outs = [], [], [], []
        for alloc in nc.m.functions[0].allocations:
            if not isinstance(alloc, mybir.MemoryLocationSet):
                continue
            name = alloc.memorylocations[0].name
            if alloc.kind == "ExternalInput":
                if name != partition_name:
                    in_names.append(name)
            elif alloc.kind == "ExternalOutput":
                out_names.append(name)
                shape = tuple(alloc.tensor_shape)
                dtype = mybir.dt.np(alloc.dtype)
                out_avals.append(jax.core.ShapedArray(shape, dtype))
                zero_outs.append(np.zeros(shape, dtype))
        n_params = len(in_names)
        in_names_all = list(in_names) + out_names + (
            [partition_name] if partition_name else [])
        donate = tuple(range(n_params, n_params + len(out_avals)))

        def _body(*args_):
            operands = list(args_)
            if partition_name is not None:
                operands.append(b2j.partition_id_tensor())
            return tuple(b2j._bass_exec_p.bind(
                *operands, out_avals=tuple(out_avals),
                in_names=tuple(in_names_all), out_names=tuple(out_names),
                lowering_input_output_aliases=(),
                sim_require_finite=True, sim_require_nnan=True, nc=nc))

        mesh = Mesh(np.asarray(jax.devices()[:n_cores]), ("core",))
        spec_in = (PartitionSpec("core"),) * (n_params + len(out_avals))
        spec_out = (PartitionSpec("core"),) * len(out_avals)
        sharded = jax.jit(
            shard_map(_body, mesh=mesh, in_specs=spec_in,
                      out_specs=spec_out, check_rep=False),
            donate_argnums=donate, keep_unused=True)
        return sharded, in_names, out_names, out_avals, zero_outs

    def fast(nc, in_maps, n_cores):
        key = (id(nc), n_cores)
        if key not in _DISPATCH:
            _DISPATCH[key] = build(nc, n_cores)
        sharded, in_names, out_names, out_avals, zero_outs = _DISPATCH[key]
        concat_in = [
            np.concatenate([np.asarray(m[name]) for m in in_maps], axis=0)
            for name in in_names]
        concat_zeros = [
            np.zeros((n_cores * z.shape[0], *z.shape[1:]), z.dtype)
            for z in zero_outs]
        out_arrs = sharded(*concat_in, *concat_zeros)
        outs_np = [np.asarray(o).reshape(n_cores, *out_avals[i].shape)
                   for i, o in enumerate(out_arrs)]
        return [{name: outs_np[i][c] for i, name in enumerate(out_names)}
                for c in range(n_cores)]

    fast._fast = True
    fast._orig = orig
    b2j.run_bass_via_pjrt = fast


def _build_program():
    nc = bacc.Bacc("TRN2", target_bir_lowering=False, debug=False,
                   num_devices=NCORES)

    def din(name, shape):
        return nc.dram_tensor(name, list(shape), FP32,
                              kind="ExternalInput").ap()

    emb = din("emb", [256, NCOL])
    aux = din("aux", [4, NCOL])
    wsl = din("wsl", [128, G4])
    waux = din("waux", [4, G4])
    wout = din("wout", [513, NT])
    c0m = din("c0m", [1, 2 * H])      # cols 0:256 fwd c0, 256:512 bwd c0
    oh = din("oh", [1, 12 * 128])     # event e -> cols [128e, 128e+128)

    feats_out = nc.dram_tensor("featsT", [NT, NF], FP32,
                               kind="ExternalOutput").ap()

    AL = mybir.AluOpType
    ACT = mybir.ActivationFunctionType

    with tile.TileContext(nc) as tc:
        import contextlib
        ctx = contextlib.ExitStack()
        with ctx:
            dram = ctx.enter_context(
                tc.tile_pool(name="dram", bufs=1, space="DRAM"))
            const = ctx.enter_context(tc.tile_pool(name="const", bufs=1))
            state = ctx.enter_context(tc.tile_pool(name="state", bufs=1))
            work = ctx.enter_context(tc.tile_pool(name="work", bufs=2))
            zp = ctx.enter_context(
                tc.tile_pool(name="zp", bufs=3, space="PSUM"))
            tp = ctx.enter_context(
                tc.tile_pool(name="tp", bufs=2, space="PSUM"))

            # ---- weight AllGather: [128,1024]/core -> [1024,1024] ----
            wg_in = dram.tile([128, G4], FP32)
            wg_out = dram.tile([8 * 128, G4], FP32)
            nc.gpsimd.dma_start(wg_in[:], wsl[:, :])
            nc.gpsimd.collective_compute(
                "AllGather", AL.bypass,
                replica_groups=[list(range(NCORES))],
                ins=[wg_in.opt()], outs=[wg_out.opt()])

            # gathered row layout: core q block at 128q; within block,
            # matrix m (wihf,whhf,wihb,whhb) rows [32m:32m+32] hold the
            # original rows [32q:32q+32].
            wt = {}
            for m, tag in enumerate(("wif", "whf", "wib", "whb")):
                t0 = const.tile([128, G4], FP32, tag=f"{tag}0")
                t1 = const.tile([128, G4], FP32, tag=f"{tag}1")
                for q in range(4):
                    nc.sync.dma_start(
                        t0[32 * q:32 * (q + 1), :],
                        wg_out[128 * q + 32 * m:128 * q + 32 * m + 32, :])
                    nc.sync.dma_start(
                        t1[32 * q:32 * (q + 1), :],
                        wg_out[128 * (q + 4) + 32 * m:128 * (q + 4) + 32 * m + 32, :])
                wt[tag] = (t0, t1)

            # ---- plain input loads ----
            ef0 = const.tile([128, NCOL], FP32, tag="ef0")
            ef1 = const.tile([128, NCOL], FP32, tag="ef1")
            nc.sync.dma_start(ef0[:], emb[0:128, :])
            nc.sync.dma_start(ef1[:], emb[128:256, :])
            auxf = const.tile([2, NCOL], FP32, tag="auxf")
            auxb = const.tile([2, NCOL], FP32, tag="auxb")
            nc.sync.dma_start(auxf[:], aux[0:2, :])
            nc.sync.dma_start(auxb[:], aux[2:4, :])
            wauxf = const.tile([2, G4], FP32, tag="wauxf")
            wauxb = const.tile([2, G4], FP32, tag="wauxb")
            nc.sync.dma_start(wauxf[:], waux[0:2, :])
            nc.sync.dma_start(wauxb[:], waux[2:4, :])
            wo = []
            for i in range(4):
                woi = const.tile([128, NT], FP32, tag=f"wo{i}")
                nc.sync.dma_start(woi[:], wout[128 * i:128 * (i + 1), :])
                wo.append(woi)
            wob = const.tile([1, NT], FP32, tag="wob")
            nc.sync.dma_start(wob[:], wout[512:513, :])
            c0t = const.tile([1, 2 * H], FP32, tag="c0t")
            nc.sync.dma_start(c0t[:], c0m[:, :])
            oht = const.tile([1, 12 * 128], FP32, tag="oht")
            nc.sync.dma_start(oht[:], oh[:, :])

            # ---- identity for PE transpose (affine_select diag) ----
            ones = const.tile([128, 128], FP32, tag="ones")
            idn = const.tile([128, 128], FP32, tag="idn")
            nc.vector.memset(ones[:], 1.0)
            nc.gpsimd.affine_select(
                out=idn[:], in_=ones[:], pattern=[[1, 128]],
                compare_op=AL.is_equal, fill=0.0,
                base=0, channel_multiplier=-1)

            # ---- c0 injection tiles: onehot(row) (x) c0[dir] ----
            inj = []
            for e in range(12):
                ps = zp.tile([128, H], FP32, tag="z")
                src = c0t[0:1, 0:H] if e < 6 else c0t[0:1, H:2 * H]
                nc.tensor.matmul(ps[:], oht[0:1, 128 * e:128 * (e + 1)], src,
                                 start=True, stop=True)
                it = const.tile([128, H], FP32, tag=f"inj{e}")
                nc.vector.tensor_copy(out=it[:], in_=ps[:])
                inj.append(it)

            # ---- persistent state ----
            hsf0 = state.tile([128, NCOL], FP32, tag="hsf0")
            hsf1 = state.tile([128, NCOL], FP32, tag="hsf1")
            hsb0 = state.tile([128, NCOL], FP32, tag="hsb0")
            hsb1 = state.tile([128, NCOL], FP32, tag="hsb1")
            cf = state.tile([128, H], FP32, tag="cf")
            cb = state.tile([128, H], FP32, tag="cb")
            for t in (hsf0, hsf1, hsb0, hsb1, cf, cb):
                nc.vector.memset(t[:], 0.0)

            def strided(tl, base):
                # cols {base + 8r, r=0..127} of a [p, 8*m] tile
                q, b = divmod(base, L)
                v = tl[:].rearrange("p (n k) -> p n k", k=L)
                return v[:, q:q + 128, b:b + 1]

            def lstm_step(s, emb_base, h_base, auxd, wauxd,
                          wih, whh, hs, c, inj_e):
                w0, w1 = wih
                g0, g1 = whh
                h0t, h1t = hs
                z = zp.tile([128, G4], FP32, tag="z")
                ktiles = [
                    (strided(ef0, emb_base), w0[:]),
                    (strided(ef1, emb_base), w1[:]),
                    (strided(auxd, emb_base), wauxd[:]),
                    (strided(h0t, h_base), g0[:]),
                    (strided(h1t, h_base), g1[:]),
                ]
                for ki, (lhs, wmat) in enumerate(ktiles):
                    first, last = ki == 0, ki == len(ktiles) - 1
                    for half in (0, 1):
                        sl = slice(512 * half, 512 * (half + 1))
                        nc.tensor.matmul(z[:, sl], lhs, wmat[:, sl],
                                         start=first, stop=last)
                # gates: sigmoid(x) = 0.5 + 0.5*tanh(0.5x)
                sg = work.tile([128, 768], FP32, tag="sg")
                gg = work.tile([128, H], FP32, tag="gg")
                nc.scalar.activation(sg[:], z[:, 0:768], ACT.Tanh, scale=0.5)
                nc.scalar.activation(gg[:], z[:, 768:1024], ACT.Tanh)
                if inj_e is not None:
                    # c0 joins the incoming state (f-gate scales it);
                    # all-zero tile on non-boundary cores -> exact no-op
                    nc.vector.tensor_tensor(out=c[:], in0=c[:],
                                            in1=inj[inj_e][:], op=AL.add)
                ig = work.tile([128, H], FP32, tag="ig")
                fg = work.tile([128, H], FP32, tag="fg")
                og = work.tile([128, H], FP32, tag="og")
                for dst, lo in ((ig, 0), (fg, H), (og, 2 * H)):
                    nc.vector.tensor_scalar(
                        out=dst[:], in0=sg[:, lo:lo + H],
                        scalar1=0.5, scalar2=0.5, op0=AL.mult, op1=AL.add)
                c1 = work.tile([128, H], FP32, tag="c1")
                c2 = work.tile([128, H], FP32, tag="c2")
                nc.vector.tensor_tensor(out=c1[:], in0=fg[:], in1=c[:],
                                        op=AL.mult)
                nc.vector.tensor_tensor(out=c2[:], in0=ig[:], in1=gg[:],
                                        op=AL.mult)
                nc.vector.tensor_tensor(out=c[:], in0=c1[:], in1=c2[:],
                                        op=AL.add)
                thc = work.tile([128, H], FP32, tag="thc")
                nc.scalar.activation(thc[:], c[:], ACT.Tanh)
                hp = work.tile([128, H], FP32, tag="hp")
                nc.vector.tensor_tensor(out=hp[:], in0=og[:], in1=thc[:],
                                        op=AL.mult)
                return hp

            for s in range(SL):
                # fwd event index e = r = (40-s)/8 -> inj rows 0..5
                inj_f = (40 - s) // 8 if s in INJ_STEPS else None
                hp_f = lstm_step(s, s + 1, s, auxf, wauxf,
                                 wt["wif"], wt["whf"], (hsf0, hsf1), cf,
                                 inj_f)
                for half, dst in ((0, hsf0), (1, hsf1)):
                    pt = tp.tile([128, 128], FP32, tag="pt")
                    nc.tensor.transpose(
                        pt[:], hp_f[:, 128 * half:128 * (half + 1)], idn[:])
                    nc.vector.tensor_copy(strided(dst, s + 1), pt[:])
                inj_b = 6 + s // 8 if s in INJ_STEPS else None
                hp_b = lstm_step(s, 2 * W + 8 - s, 2 * W + 9 - s,
                                 auxb, wauxb,
                                 wt["wib"], wt["whb"], (hsb0, hsb1), cb,
                                 inj_b)
                for half, dst in ((0, hsb0), (1, hsb1)):
                    pt = tp.tile([128, 128], FP32, tag="pt")
                    nc.tensor.transpose(
                        pt[:], hp_b[:, 128 * half:128 * (half + 1)], idn[:])
                    nc.vector.tensor_copy(strided(dst, 2 * W + 8 - s), pt[:])

            # ---- bulk feats: featsT[i, tau] ; hs col = tau + COFF ----
            fsb = state.tile([NT, NF], FP32, tag="fsb")
            fstep = 512
            for f0 in range(0, NF, fstep):
                n = min(fstep, NF - f0)
                fp = zp.tile([NT, n], FP32, tag="z")
                c0_, c1_ = COFF + f0, COFF + f0 + n
                nc.tensor.matmul(fp[:], wo[0][:], hsf0[:, c0_:c1_],
                                 start=True, stop=False)
                nc.tensor.matmul(fp[:], wo[1][:], hsf1[:, c0_:c1_],
                                 start=False, stop=False)
                nc.tensor.matmul(fp[:], wo[2][:], hsb0[:, c0_:c1_],
                                 start=False, stop=False)
                nc.tensor.matmul(fp[:], wo[3][:], hsb1[:, c0_:c1_],
                                 start=False, stop=False)
                nc.tensor.matmul(fp[:], wob[:], auxf[0:1, c0_:c1_],
                                 start=False, stop=True)
                nc.vector.tensor_copy(out=fsb[:, f0:f0 + n], in_=fp[:])
            nc.sync.dma_start(feats_out[:, :], fsb[:])

    nc.compile()
    return nc


def _prep_core(k, sentence, embed, wihf_t, whhf_t, wihb_t, whhb_t,
               b_f, b_b, wh0_f, wh0_b, W_out, b_out, c0):
    s_k = OWN * k
    t = s_k + np.arange(NCOL) - COFF
    valid = (t >= 0) & (t < T)
    tv = np.clip(t, 0, T - 1)
    emb = np.ascontiguousarray(embed[sentence[tv]].T)   # [256, NCOL]
    emb[:, ~valid] = 0.0

    aux = np.zeros((4, NCOL), dtype=np.float32)
    aux[0] = valid
    aux[1] = (t == 0)
    aux[2] = valid
    aux[3] = (t == T - 1)

    wsl = np.concatenate([m[32 * k:32 * (k + 1)] for m in
                          (wihf_t, whhf_t, wihb_t, whhb_t)], axis=0)

    waux = np.stack([b_f, wh0_f, b_b, wh0_b]).astype(np.float32)

    wout = np.zeros((513, NT), dtype=np.float32)
    wout[0:256] = W_out[:, 0:256].T
    wout[256:512] = W_out[:, 256:512].T
    wout[512] = b_out

    c0m = np.ascontiguousarray(c0.astype(np.float32).reshape(1, 2 * H))

    oh = np.zeros((1, 12 * 128), dtype=np.float32)
    if k == 0:
        for e in range(6):
            oh[0, 128 * e + e] = 1.0
    if k == NCORES - 1:
        for j in range(6):
            oh[0, 128 * (6 + j) + 122 + j] = 1.0

    return {"emb": emb, "aux": aux, "wsl": np.ascontiguousarray(wsl),
            "waux": waux, "wout": wout, "c0m": c0m, "oh": oh}


def _host_viterbi(feats, trans):
    """Exact fp32 replica of the reference Viterbi scan + backtrack."""
    Tn = feats.shape[0]
    feats = np.ascontiguousarray(feats, dtype=np.float32)
    trans = np.ascontiguousarray(trans, dtype=np.float32)
    fv = np.full(NT, NEG, np.float32)
    fv[START_IX] = 0.0
    fv_prev = np.empty((Tn, NT), np.float32)
    for t in range(Tn):
        fv_prev[t] = fv
        temp = (fv[None, :] + feats[t][:, None]) + trans
        fv = temp.max(1)
    # vectorized backpointer replay (same fp op order per element)
    temp_all = (fv_prev[:, None, :] + feats[:, :, None]) + trans[None]
    bps = temp_all.argmax(2)                            # [Tn, 16]
    fv = fv + trans[:, STOP_IX]
    idc = int(fv.argmax())
    path = np.empty(Tn, np.int64)
    path[Tn - 1] = idc
    for t in range(Tn - 2, -1, -1):
        path[t] = bps[t + 1][path[t + 1]]
    return path


def kernel(sentence, embed, w_ih_f, w_hh_f, b_ih_f, b_hh_f,
           w_ih_b, w_hh_b, b_ih_b, b_hh_b, W_out, b_out,
           transition, h0, c0):
    global _COMPILED
    sentence = np.asarray(sentence).astype(np.int64)
    embed = np.asarray(embed, dtype=np.float32)
    args = [np.asarray(a, dtype=np.float32) for a in
            (w_ih_f, w_hh_f, b_ih_f, b_hh_f, w_ih_b, w_hh_b, b_ih_b, b_hh_b,
             W_out, b_out, transition, h0, c0)]
    (w_ih_f, w_hh_f, b_ih_f, b_hh_f, w_ih_b, w_hh_b, b_ih_b, b_hh_b,
     W_out, b_out, transition, h0, c0) = args

    wihf_t = np.ascontiguousarray(w_ih_f.T[:, GATE_PERM])
    whhf_t = np.ascontiguousarray(w_hh_f.T[:, GATE_PERM])
    wihb_t = np.ascontiguousarray(w_ih_b.T[:, GATE_PERM])
    whhb_t = np.ascontiguousarray(w_hh_b.T[:, GATE_PERM])
    b_f = (b_ih_f + b_hh_f)[GATE_PERM]
    b_b = (b_ih_b + b_hh_b)[GATE_PERM]
    wh0_f = (w_hh_f @ h0[0])[GATE_PERM]
    wh0_b = (w_hh_b @ h0[1])[GATE_PERM]

    if _COMPILED is None:
        _install_fast_pjrt()
        _COMPILED = _build_program()
    nc = _COMPILED

    in_maps = [
        _prep_core(k, sentence, embed, wihf_t, whhf_t, wihb_t, whhb_t,
                   b_f, b_b, wh0_f, wh0_b, W_out, b_out, c0)
        for k in range(NCORES)
    ]

    _t0 = _time.perf_counter()
    res = run_bass_kernel_spmd(nc, in_maps, core_ids=list(range(NCORES)),
                               trace=False)
    kernel.last_dispatch_wall_ns = int((_time.perf_counter() - _t0) * 1e9)
    kernel.last_exec_time_ns = getattr(res, "exec_time_ns", None)

    feats_full = np.empty((T, NT), dtype=np.float32)
    for k in range(NCORES):
        feats_full[OWN * k:OWN * (k + 1)] = res.results[k]["featsT"].T
    if os.environ.get("KERNEL_DEBUG_FEATS"):
        np.save("/tmp/feats_device.npy", feats_full)

    path = _host_viterbi(feats_full, transition)
    return path.astype(np.int32)


# revision 23
# speedup vs baseline: 2.2310x; 1.1083x over previous
"""BiLSTM-CRF Trainium2 kernel (8 NeuronCores, SPMD).

Strategy (v2 — fp32 end-to-end, upload-lean):
 - Data-parallel over the sequence: core k owns tokens [1024k, 1024k+1024).
 - Chunked-warmup LSTM: 128 rows x 8 owned tokens each, W=40 warmup steps
   run in lockstep (state reconvergence ~1e-10; boundary rows at t=0 /
   t=T-1 get exact h0/c0 injection, so no approximation there at all).
 - Everything fp32 on device: embeddings, weights, h/c state, PSUM.
   sigmoid computed as 0.5 + 0.5*tanh(0.5 z) (device tanh spline is
   ~4 ULP; measured 4.4e-8 abs err for the composite, ~20x better than
   the direct sigmoid spline).
 - Upload diet (axon tunnel ~85 MB/s, ~60 ms fixed): one shared fp32
   embedding window per core serves both directions (1.14 MB); the
   4 MB of LSTM weights are uploaded as 1/8 row-slices per core and
   AllGathered on device; identity generated with affine_select;
   c0 injection tiles built on device from a 2x256 vector.
 - feats.T = W_out @ [h_f; h_b] + b_out in bulk; [16,1024] fp32 out/core.
 - Host: exact fp32 replica of the reference Viterbi scan (same op
   order as jax CPU; validated bit-exact) + vectorized backpointer
   replay + backtrack.
"""

import os
import sys
import time as _time

import numpy as np

sys.path.insert(0, "/opt/trn_rl_repo")

import concourse.bass as bass  # noqa: E402
import concourse.tile as tile  # noqa: E402
from concourse import bacc, mybir  # noqa: E402
from concourse.bass_utils import run_bass_kernel_spmd  # noqa: E402

# ---- problem constants (hardcoded per the task contract) ----
T = 8192
VOCAB = 100000
EMBED = 256
H = 256            # per-direction hidden
G4 = 1024
NT = 16
START_IX = 14
STOP_IX = 15
NEG = -10000.0
NCORES = 8
OWN = T // NCORES  # 1024

# chunked-warmup geometry
L = 8              # owned tokens per row
W = 40             # LSTM warmup steps per row
SL = L + W         # 48 lockstep steps
NCOL = 8 * 139     # 1112 emb/hs columns; col c <-> t_rel = c - (W+1)
COFF = W + 1       # 41
NF = 1024          # feats per core

FP32 = mybir.dt.float32

# gate reorder: torch [i,f,g,o] -> device [i,f,o,g] (sigmoid block 0:768)
GATE_PERM = np.concatenate([
    np.arange(0, 256), np.arange(256, 512), np.arange(768, 1024),
    np.arange(512, 768)])

# injection events: fwd (core 0) rows 0..5 consume t=0 at step 40-8r;
# bwd (core 7) rows 122..127 consume t=T-1 at step 8r-976.
INJ_STEPS = (0, 8, 16, 24, 32, 40)

_COMPILED = None
_DISPATCH = {}


def _install_fast_pjrt():
    """Cache the jit(shard_map(bass_exec)) callable across calls.

    ``bass2jax.run_bass_via_pjrt`` rebuilds the jit wrapper on every
    invocation (a fresh closure forces a full jax retrace, ~130 ms) and
    materializes each sharded output once per core (redundant D2H
    fetches).  Execution still flows unchanged through
    ``run_bass_kernel_spmd`` -> ``_bass_exec_p`` -> PJRT; this only
    memoizes the host-side dispatch plumbing.
    """
    from concourse import bass2jax as b2j

    if getattr(b2j.run_bass_via_pjrt, "_fast", False):
        return
    orig = b2j.run_bass_via_pjrt

    import jax
    from jax.sharding import Mesh, PartitionSpec
    from jax.experimental.shard_map import shard_map

    def build(nc, n_cores):
        b2j.install_neuronx_cc_hook()
        partition_name = (nc.partition_id_tensor.name
                          if nc.partition_id_tensor else None)
        in_names, out_names, out_avals, zero_outs = [], [], [], []
        for alloc in nc.m.functions[0].allocations:
            if not isinstance(alloc, mybir.MemoryLocationSet):
                continue
            name = alloc.memorylocations[0].name
            if alloc.kind == "ExternalInput":
                if name != partition_name:
                    in_names.append(name)
            elif alloc.kind == "ExternalOutput":
                out_names.append(name)
                shape = tuple(alloc.tensor_shape)
                dtype = mybir.dt.np(alloc.dtype)
                out_avals.append(jax.core.ShapedArray(shape, dtype))
                zero_outs.append(np.zeros(shape, dtype))
        n_params = len(in_names)
        in_names_all = list(in_names) + out_names + (
            [partition_name] if partition_name else [])
        donate = tuple(range(n_params, n_params + len(out_avals)))

        def _body(*args_):
            operands = list(args_)
            if partition_name is not None:
                operands.append(b2j.partition_id_tensor())
            return tuple(b2j._bass_exec_p.bind(
                *operands, out_avals=tuple(out_avals),
                in_names=tuple(in_names_all), out_names=tuple(out_names),
                lowering_input_output_aliases=(),
                sim_require_finite=True, sim_require_nnan=True, nc=nc))

        mesh = Mesh(np.asarray(jax.devices()[:n_cores]), ("core",))
        spec_in = (PartitionSpec("core"),) * (n_params + len(out_avals))
        spec_out = (PartitionSpec("core"),) * len(out_avals)
        sharded = jax.jit(
            shard_map(_body, mesh=mesh, in_specs=spec_in,
                      out_specs=spec_out, check_rep=False),
            donate_argnums=donate, keep_unused=True)
        return sharded, in_names, out_names, out_avals, zero_outs

    def fast(nc, in_maps, n_cores):
        key = (id(nc), n_cores)
        if key not in _DISPATCH:
            _DISPATCH[key] = build(nc, n_cores)
        sharded, in_names, out_names, out_avals, zero_outs = _DISPATCH[key]
        concat_in = [
            np.concatenate([np.asarray(m[name]) for m in in_maps], axis=0)
            for name in in_names]
        concat_zeros = [
            np.zeros((n_cores * z.shape[0], *z.shape[1:]), z.dtype)
            for z in zero_outs]
        out_arrs = sharded(*concat_in, *concat_zeros)
        outs_np = [np.asarray(o).reshape(n_cores, *out_avals[i].shape)
                   for i, o in enumerate(out_arrs)]
        return [{name: outs_np[i][c] for i, name in enumerate(out_names)}
                for c in range(n_cores)]

    fast._fast = True
    fast._orig = orig
    b2j.run_bass_via_pjrt = fast


def _build_program():
    nc = bacc.Bacc("TRN2", target_bir_lowering=False, debug=False,
                   num_devices=NCORES)

    def din(name, shape, dt=FP32):
        return nc.dram_tensor(name, list(shape), dt,
                              kind="ExternalInput").ap()

    # emb fp32 = 2^-12 * embh (int16) + 2^-20 * embl (int8)
    embh = din("embh", [256, NCOL], mybir.dt.int16)
    embl = din("embl", [256, NCOL], mybir.dt.int8)
    aux = din("aux", [4, NCOL])
    wsl = din("wsl", [128, G4])
    waux = din("waux", [4, G4])
    wout = din("wout", [513, NT])
    c0m = din("c0m", [1, 2 * H])      # cols 0:256 fwd c0, 256:512 bwd c0
    oh = din("oh", [1, 12 * 128])     # event e -> cols [128e, 128e+128)

    feats_out = nc.dram_tensor("featsT", [NT, NF], FP32,
                               kind="ExternalOutput").ap()

    AL = mybir.AluOpType
    ACT = mybir.ActivationFunctionType

    with tile.TileContext(nc) as tc:
        import contextlib
        ctx = contextlib.ExitStack()
        with ctx:
            dram = ctx.enter_context(
                tc.tile_pool(name="dram", bufs=1, space="DRAM"))
            const = ctx.enter_context(tc.tile_pool(name="const", bufs=1))
            state = ctx.enter_context(tc.tile_pool(name="state", bufs=1))
            work = ctx.enter_context(tc.tile_pool(name="work", bufs=2))
            zp = ctx.enter_context(
                tc.tile_pool(name="zp", bufs=3, space="PSUM"))
            tp = ctx.enter_context(
                tc.tile_pool(name="tp", bufs=2, space="PSUM"))

            # ---- weight AllGather: [128,1024]/core -> [1024,1024] ----
            wg_in = dram.tile([128, G4], FP32)
            wg_out = dram.tile([8 * 128, G4], FP32)
            nc.gpsimd.dma_start(wg_in[:], wsl[:, :])
            nc.gpsimd.collective_compute(
                "AllGather", AL.bypass,
                replica_groups=[list(range(NCORES))],
                ins=[wg_in.opt()], outs=[wg_out.opt()])

            # gathered row layout: core q block at 128q; within block,
            # matrix m (wihf,whhf,wihb,whhb) rows [32m:32m+32] hold the
            # original rows [32q:32q+32].
            wt = {}
            for m, tag in enumerate(("wif", "whf", "wib", "whb")):
                t0 = const.tile([128, G4], FP32, tag=f"{tag}0")
                t1 = const.tile([128, G4], FP32, tag=f"{tag}1")
                for q in range(4):
                    nc.sync.dma_start(
                        t0[32 * q:32 * (q + 1), :],
                        wg_out[128 * q + 32 * m:128 * q + 32 * m + 32, :])
                    nc.sync.dma_start(
                        t1[32 * q:32 * (q + 1), :],
                        wg_out[128 * (q + 4) + 32 * m:128 * (q + 4) + 32 * m + 32, :])
                wt[tag] = (t0, t1)

            # ---- plain input loads ----
            ef0 = const.tile([128, NCOL], FP32, tag="ef0")
            ef1 = const.tile([128, NCOL], FP32, tag="ef1")
            etmp = const.tile([128, NCOL], FP32, tag="etmp")
            for half, ef in ((0, ef0), (1, ef1)):
                rows = slice(128 * half, 128 * (half + 1))
                ehi = const.tile([128, NCOL], mybir.dt.int16,
                                 tag=f"ehi{half}")
                elo = const.tile([128, NCOL], mybir.dt.int8,
                                 tag=f"elo{half}")
                nc.sync.dma_start(ehi[:], embh[rows, :])
                nc.sync.dma_start(elo[:], embl[rows, :])
                nc.vector.tensor_scalar(out=ef[:], in0=ehi[:],
                                        scalar1=float(2.0 ** -12),
                                        scalar2=None, op0=AL.mult)
                nc.vector.tensor_scalar(out=etmp[:], in0=elo[:],
                                        scalar1=float(2.0 ** -20),
                                        scalar2=None, op0=AL.mult)
                nc.vector.tensor_tensor(out=ef[:], in0=ef[:], in1=etmp[:],
                                        op=AL.add)
            auxf = const.tile([2, NCOL], FP32, tag="auxf")
            auxb = const.tile([2, NCOL], FP32, tag="auxb")
            nc.sync.dma_start(auxf[:], aux[0:2, :])
            nc.sync.dma_start(auxb[:], aux[2:4, :])
            wauxf = const.tile([2, G4], FP32, tag="wauxf")
            wauxb = const.tile([2, G4], FP32, tag="wauxb")
            nc.sync.dma_start(wauxf[:], waux[0:2, :])
            nc.sync.dma_start(wauxb[:], waux[2:4, :])
            wo = []
            for i in range(4):
                woi = const.tile([128, NT], FP32, tag=f"wo{i}")
                nc.sync.dma_start(woi[:], wout[128 * i:128 * (i + 1), :])
                wo.append(woi)
            wob = const.tile([1, NT], FP32, tag="wob")
            nc.sync.dma_start(wob[:], wout[512:513, :])
            c0t = const.tile([1, 2 * H], FP32, tag="c0t")
            nc.sync.dma_start(c0t[:], c0m[:, :])
            oht = const.tile([1, 12 * 128], FP32, tag="oht")
            nc.sync.dma_start(oht[:], oh[:, :])

            # ---- identity for PE transpose (affine_select diag) ----
            ones = const.tile([128, 128], FP32, tag="ones")
            idn = const.tile([128, 128], FP32, tag="idn")
            nc.vector.memset(ones[:], 1.0)
            nc.gpsimd.affine_select(
                out=idn[:], in_=ones[:], pattern=[[1, 128]],
                compare_op=AL.is_equal, fill=0.0,
                base=0, channel_multiplier=-1)

            # ---- c0 injection tiles: onehot(row) (x) c0[dir] ----
            inj = []
            for e in range(12):
                ps = zp.tile([128, H], FP32, tag="z")
                src = c0t[0:1, 0:H] if e < 6 else c0t[0:1, H:2 * H]
                nc.tensor.matmul(ps[:], oht[0:1, 128 * e:128 * (e + 1)], src,
                                 start=True, stop=True)
                it = const.tile([128, H], FP32, tag=f"inj{e}")
                nc.vector.tensor_copy(out=it[:], in_=ps[:])
                inj.append(it)

            # ---- persistent state ----
            hsf0 = state.tile([128, NCOL], FP32, tag="hsf0")
            hsf1 = state.tile([128, NCOL], FP32, tag="hsf1")
            hsb0 = state.tile([128, NCOL], FP32, tag="hsb0")
            hsb1 = state.tile([128, NCOL], FP32, tag="hsb1")
            cf = state.tile([128, H], FP32, tag="cf")
            cb = state.tile([128, H], FP32, tag="cb")
            for t in (hsf0, hsf1, hsb0, hsb1, cf, cb):
                nc.vector.memset(t[:], 0.0)

            def strided(tl, base):
                # cols {base + 8r, r=0..127} of a [p, 8*m] tile
                q, b = divmod(base, L)
                v = tl[:].rearrange("p (n k) -> p n k", k=L)
                return v[:, q:q + 128, b:b + 1]

            def lstm_step(s, emb_base, h_base, auxd, wauxd,
                          wih, whh, hs, c, inj_e):
                w0, w1 = wih
                g0, g1 = whh
                h0t, h1t = hs
                z = zp.tile([128, G4], FP32, tag="z")
                ktiles = [
                    (strided(ef0, emb_base), w0[:]),
                    (strided(ef1, emb_base), w1[:]),
                    (strided(auxd, emb_base), wauxd[:]),
                    (strided(h0t, h_base), g0[:]),
                    (strided(h1t, h_base), g1[:]),
                ]
                for ki, (lhs, wmat) in enumerate(ktiles):
                    first, last = ki == 0, ki == len(ktiles) - 1
                    for half in (0, 1):
                        sl = slice(512 * half, 512 * (half + 1))
                        nc.tensor.matmul(z[:, sl], lhs, wmat[:, sl],
                                         start=first, stop=last)
                # gates: sigmoid(x) = 0.5 + 0.5*tanh(0.5x)
                sg = work.tile([128, 768], FP32, tag="sg")
                gg = work.tile([128, H], FP32, tag="gg")
                nc.scalar.activation(sg[:], z[:, 0:768], ACT.Tanh, scale=0.5)
                nc.scalar.activation(gg[:], z[:, 768:1024], ACT.Tanh)
                if inj_e is not None:
                    # c0 joins the incoming state (f-gate scales it);
                    # all-zero tile on non-boundary cores -> exact no-op
                    nc.vector.tensor_tensor(out=c[:], in0=c[:],
                                            in1=inj[inj_e][:], op=AL.add)
                ig = work.tile([128, H], FP32, tag="ig")
                fg = work.tile([128, H], FP32, tag="fg")
                og = work.tile([128, H], FP32, tag="og")
                for dst, lo in ((ig, 0), (fg, H), (og, 2 * H)):
                    nc.vector.tensor_scalar(
                        out=dst[:], in0=sg[:, lo:lo + H],
                        scalar1=0.5, scalar2=0.5, op0=AL.mult, op1=AL.add)
                c1 = work.tile([128, H], FP32, tag="c1")
                c2 = work.tile([128, H], FP32, tag="c2")
                nc.vector.tensor_tensor(out=c1[:], in0=fg[:], in1=c[:],
                                        op=AL.mult)
                nc.vector.tensor_tensor(out=c2[:], in0=ig[:], in1=gg[:],
                                        op=AL.mult)
                nc.vector.tensor_tensor(out=c[:], in0=c1[:], in1=c2[:],
                                        op=AL.add)
                thc = work.tile([128, H], FP32, tag="thc")
                nc.scalar.activation(thc[:], c[:], ACT.Tanh)
                hp = work.tile([128, H], FP32, tag="hp")
                nc.vector.tensor_tensor(out=hp[:], in0=og[:], in1=thc[:],
                                        op=AL.mult)
                return hp

            for s in range(SL):
                # fwd event index e = r = (40-s)/8 -> inj rows 0..5
                inj_f = (40 - s) // 8 if s in INJ_STEPS else None
                hp_f = lstm_step(s, s + 1, s, auxf, wauxf,
                                 wt["wif"], wt["whf"], (hsf0, hsf1), cf,
                                 inj_f)
                for half, dst in ((0, hsf0), (1, hsf1)):
                    pt = tp.tile([128, 128], FP32, tag="pt")
                    nc.tensor.transpose(
                        pt[:], hp_f[:, 128 * half:128 * (half + 1)], idn[:])
                    nc.vector.tensor_copy(strided(dst, s + 1), pt[:])
                inj_b = 6 + s // 8 if s in INJ_STEPS else None
                hp_b = lstm_step(s, 2 * W + 8 - s, 2 * W + 9 - s,
                                 auxb, wauxb,
                                 wt["wib"], wt["whb"], (hsb0, hsb1), cb,
                                 inj_b)
                for half, dst in ((0, hsb0), (1, hsb1)):
                    pt = tp.tile([128, 128], FP32, tag="pt")
                    nc.tensor.transpose(
                        pt[:], hp_b[:, 128 * half:128 * (half + 1)], idn[:])
                    nc.vector.tensor_copy(strided(dst, 2 * W + 8 - s), pt[:])

            # ---- bulk feats: featsT[i, tau] ; hs col = tau + COFF ----
            fsb = state.tile([NT, NF], FP32, tag="fsb")
            fstep = 512
            for f0 in range(0, NF, fstep):
                n = min(fstep, NF - f0)
                fp = zp.tile([NT, n], FP32, tag="z")
                c0_, c1_ = COFF + f0, COFF + f0 + n
                nc.tensor.matmul(fp[:], wo[0][:], hsf0[:, c0_:c1_],
                                 start=True, stop=False)
                nc.tensor.matmul(fp[:], wo[1][:], hsf1[:, c0_:c1_],
                                 start=False, stop=False)
                nc.tensor.matmul(fp[:], wo[2][:], hsb0[:, c0_:c1_],
                                 start=False, stop=False)
                nc.tensor.matmul(fp[:], wo[3][:], hsb1[:, c0_:c1_],
                                 start=False, stop=False)
                nc.tensor.matmul(fp[:], wob[:], auxf[0:1, c0_:c1_],
                                 start=False, stop=True)
                nc.vector.tensor_copy(out=fsb[:, f0:f0 + n], in_=fp[:])
            nc.sync.dma_start(feats_out[:, :], fsb[:])

    nc.compile()
    return nc


def _prep_core(k, sentence, embed, wihf_t, whhf_t, wihb_t, whhb_t,
               b_f, b_b, wh0_f, wh0_b, W_out, b_out, c0):
    s_k = OWN * k
    t = s_k + np.arange(NCOL) - COFF
    valid = (t >= 0) & (t < T)
    tv = np.clip(t, 0, T - 1)
    emb = np.ascontiguousarray(embed[sentence[tv]].T)   # [256, NCOL]
    emb[:, ~valid] = 0.0
    # 3-byte split: emb = 2^-12*hi + 2^-20*lo (+ residual < 2^-21)
    hi = np.rint(emb * 4096.0)
    np.clip(hi, -32768, 32767, out=hi)
    lo = np.rint((emb - hi * (2.0 ** -12)) * (2.0 ** 20))
    np.clip(lo, -128, 127, out=lo)
    embh_q = hi.astype(np.int16)
    embl_q = lo.astype(np.int8)

    aux = np.zeros((4, NCOL), dtype=np.float32)
    aux[0] = valid
    aux[1] = (t == 0)
    aux[2] = valid
    aux[3] = (t == T - 1)

    wsl = np.concatenate([m[32 * k:32 * (k + 1)] for m in
                          (wihf_t, whhf_t, wihb_t, whhb_t)], axis=0)

    waux = np.stack([b_f, wh0_f, b_b, wh0_b]).astype(np.float32)

    wout = np.zeros((513, NT), dtype=np.float32)
    wout[0:256] = W_out[:, 0:256].T
    wout[256:512] = W_out[:, 256:512].T
    wout[512] = b_out

    c0m = np.ascontiguousarray(c0.astype(np.float32).reshape(1, 2 * H))

    oh = np.zeros((1, 12 * 128), dtype=np.float32)
    if k == 0:
        for e in range(6):
            oh[0, 128 * e + e] = 1.0
    if k == NCORES - 1:
        for j in range(6):
            oh[0, 128 * (6 + j) + 122 + j] = 1.0

    return {"embh": embh_q, "embl": embl_q, "aux": aux,
            "wsl": np.ascontiguousarray(wsl),
            "waux": waux, "wout": wout, "c0m": c0m, "oh": oh}


def _host_viterbi(feats, trans):
    """Exact fp32 replica of the reference Viterbi scan + backtrack."""
    Tn = feats.shape[0]
    feats = np.ascontiguousarray(feats, dtype=np.float32)
    trans = np.ascontiguousarray(trans, dtype=np.float32)
    fv = np.full(NT, NEG, np.float32)
    fv[START_IX] = 0.0
    fv_prev = np.empty((Tn, NT), np.float32)
    for t in range(Tn):
        fv_prev[t] = fv
        temp = (fv[None, :] + feats[t][:, None]) + trans
        fv = temp.max(1)
    # vectorized backpointer replay (same fp op order per element)
    temp_all = (fv_prev[:, None, :] + feats[:, :, None]) + trans[None]
    bps = temp_all.argmax(2)                            # [Tn, 16]
    fv = fv + trans[:, STOP_IX]
    idc = int(fv.argmax())
    path = np.empty(Tn, np.int64)
    path[Tn - 1] = idc
    for t in range(Tn - 2, -1, -1):
        path[t] = bps[t + 1][path[t + 1]]
    return path


def kernel(sentence, embed, w_ih_f, w_hh_f, b_ih_f, b_hh_f,
           w_ih_b, w_hh_b, b_ih_b, b_hh_b, W_out, b_out,
           transition, h0, c0):
    global _COMPILED
    sentence = np.asarray(sentence).astype(np.int64)
    embed = np.asarray(embed, dtype=np.float32)
    args = [np.asarray(a, dtype=np.float32) for a in
            (w_ih_f, w_hh_f, b_ih_f, b_hh_f, w_ih_b, w_hh_b, b_ih_b, b_hh_b,
             W_out, b_out, transition, h0, c0)]
    (w_ih_f, w_hh_f, b_ih_f, b_hh_f, w_ih_b, w_hh_b, b_ih_b, b_hh_b,
     W_out, b_out, transition, h0, c0) = args

    wihf_t = np.ascontiguousarray(w_ih_f.T[:, GATE_PERM])
    whhf_t = np.ascontiguousarray(w_hh_f.T[:, GATE_PERM])
    wihb_t = np.ascontiguousarray(w_ih_b.T[:, GATE_PERM])
    whhb_t = np.ascontiguousarray(w_hh_b.T[:, GATE_PERM])
    b_f = (b_ih_f + b_hh_f)[GATE_PERM]
    b_b = (b_ih_b + b_hh_b)[GATE_PERM]
    wh0_f = (w_hh_f @ h0[0])[GATE_PERM]
    wh0_b = (w_hh_b @ h0[1])[GATE_PERM]

    if _COMPILED is None:
        _install_fast_pjrt()
        _COMPILED = _build_program()
    nc = _COMPILED

    in_maps = [
        _prep_core(k, sentence, embed, wihf_t, whhf_t, wihb_t, whhb_t,
                   b_f, b_b, wh0_f, wh0_b, W_out, b_out, c0)
        for k in range(NCORES)
    ]

    _t0 = _time.perf_counter()
    res = run_bass_kernel_spmd(nc, in_maps, core_ids=list(range(NCORES)),
                               trace=False)
    kernel.last_dispatch_wall_ns = int((_time.perf_counter() - _t0) * 1e9)
    kernel.last_exec_time_ns = getattr(res, "exec_time_ns", None)

    feats_full = np.empty((T, NT), dtype=np.float32)
    for k in range(NCORES):
        feats_full[OWN * k:OWN * (k + 1)] = res.results[k]["featsT"].T
    if os.environ.get("KERNEL_DEBUG_FEATS"):
        np.save("/tmp/feats_device.npy", feats_full)

    path = _host_viterbi(feats_full, transition)
    return path.astype(np.int32)
